# revision 13
# baseline (speedup 1.0000x reference)
"""LLaMA causal self-attention, 8-way head-tensor-parallel Trainium2 Bass kernel.

Sharding: each of 8 cores computes 4 query heads + its 1 KV head-group
(Wq/Wk/Wv column-sharded), plus a row-shard of Wo producing a partial
(S, DIM) output; partials are summed on the host (the all-reduce of the
row-sharded Wo matmul).

Device layout notes:
  - x is passed pre-transposed (xT, [DIM, S]) so projection matmuls contract
    over partitions without on-device transposes.
  - q/k channel order is host-permuted per head to [evens, odds] so RoPE pair
    mixing becomes 32-row block ops (partition-aligned; the pair-swap operand
    is built with SBUF->SBUF DMAs).
  - scores are computed transposed (scoresT[k, q]) so exp(P) feeds P@V as the
    moving operand with k on partitions; a ones-column appended to v yields
    the softmax row-sums as psum row 64.
  - softmax normalization: reciprocal of sums row, rank-1 (K=1) PE matmul to
    broadcast it across 64 partitions, then one DVE multiply.
"""

import numpy as np

import concourse.bass as bass
import concourse.mybir as mybir
import concourse.tile as tile
from contextlib import nullcontext
from concourse import bacc
from concourse.bass import ts, ds
from concourse.bass_utils import run_bass_kernel_spmd
from concourse.masks import make_identity

F32 = mybir.dt.float32
F32R = mybir.dt.float32r

S = 2048
DIM = 2048
H = 32
KVH = 8
D = 64
NCORES = 8
HQ = H // NCORES          # 4 q heads per core
CQ = HQ * D               # 256 q cols per core
ST = 256                  # s-tile width in QKV projection phase
QT = 512                  # q-tile width in attention
NKT = S // 128            # 16 key tiles
NDT = DIM // 128          # 16 contraction tiles for projections
NST = S // ST             # 8 projection s-tiles
NQT = S // QT             # 4 attention q-tiles


def r(ap):
    return ap.bitcast(F32R)


def _build(causal: bool, use_mask: bool):
    nc = bacc.Bacc("TRN2", target_bir_lowering=False, debug=False,
                   num_devices=NCORES, name="llama_attn")
    xT = nc.dram_tensor("xT", [DIM, S], F32R, kind="ExternalInput")
    wq = nc.dram_tensor("wq", [DIM, CQ], F32R, kind="ExternalInput")
    wkv = nc.dram_tensor("wkv", [DIM, 128], F32R, kind="ExternalInput")
    wo = nc.dram_tensor("wo", [CQ, DIM], F32R, kind="ExternalInput")
    bqd = nc.dram_tensor("bq", [CQ], F32, kind="ExternalInput")
    bkvd = nc.dram_tensor("bkv", [128], F32, kind="ExternalInput")
    ccd = nc.dram_tensor("cc", [128, S], F32, kind="ExternalInput")
    ssd = nc.dram_tensor("ssgn", [128, S], F32, kind="ExternalInput")
    trid = nc.dram_tensor("trimask", [512, 512], F32, kind="ExternalInput")
    onesd = nc.dram_tensor("ones", [128], F32R, kind="ExternalInput")
    if use_mask:
        maskt = nc.dram_tensor("maskt", [S, S], F32, kind="ExternalInput")
    partial = nc.dram_tensor("partial", [S, DIM], F32, kind="ExternalOutput")

    with tile.TileContext(nc) as tc:
        with tc.tile_pool(name="persist", bufs=1) as pp:
            wq_sb = pp.tile([128, NDT, CQ], F32R)
            nc.sync.dma_start(wq_sb[:], wq.ap().rearrange("(a p) c -> p a c", p=128))
            wkv_sb = pp.tile([128, NDT, 128], F32R)
            nc.sync.dma_start(wkv_sb[:], wkv.ap().rearrange("(a p) c -> p a c", p=128))
            wo_sb = pp.tile([128, 2, DIM], F32R)
            nc.sync.dma_start(wo_sb[:], wo.ap().rearrange("(a p) e -> p a e", p=128))
            cc_sb = pp.tile([128, S], F32)
            nc.sync.dma_start(cc_sb[:], ccd[:])
            ss_sb = pp.tile([128, S], F32)
            nc.sync.dma_start(ss_sb[:], ssd[:])
            bq_sb = pp.tile([128, 2], F32)
            nc.sync.dma_start(bq_sb[:], bqd.ap().rearrange("(t p) -> p t", p=128))
            bkv_sb = pp.tile([128, 1], F32)
            nc.sync.dma_start(bkv_sb[:], bkvd.ap()[:, None])
            tri_sb = pp.tile([128, 4, 512], F32)
            ones65 = pp.tile([65, 128], F32R, name="ones65")
            nc.gpsimd.dma_start(ones65[64:65, :], onesd.ap()[None, :])
            nc.sync.dma_start(tri_sb[:], trid.ap().rearrange("(v p) q -> p v q", p=128))
            ident = pp.tile([64, 64], F32)
            make_identity(nc, ident[:])

            # persistent activations
            qT_raw = [pp.tile([128, S], F32, tag=f"qtraw{i}", name=f"qtraw{i}") for i in range(2)]
            kv_raw = pp.tile([128, S], F32)
            qT = [pp.tile([128, S], F32R, tag=f"qt{i}", name=f"qt{i}") for i in range(2)]
            kT = pp.tile([128, S], F32R)          # rows 0:64 and 64:128 both = roped k
            v_sb = pp.tile([128, NKT, 65], F32R)  # [kpos, ktile, ch + ones]
            nc.gpsimd.dma_start(v_sb[:, :, 64:65],
                                onesd.ap()[:, None, None].to_broadcast((128, NKT, 1)))
            attn = [pp.tile([128, S], F32R, tag=f"attn{i}", name=f"attn{i}") for i in range(2)]

            # ---------------- Phase 1a: QKV projections ----------------
            with tc.tile_pool(name="xstream", bufs=2) as xp, \
                 tc.tile_pool(name="qkvps", bufs=2, space="PSUM") as qps:
                for st in range(NST):
                    ssl = ts(st, ST)
                    xts = []
                    for kt in range(NDT):
                        xt = xp.tile([128, ST], F32R, tag=f"xt{kt}")
                        nc.sync.dma_start(xt[:], xT[ds(kt * 128, 128), ssl])
                        xts.append(xt)
                    pq0 = qps.tile([128, ST], F32, tag="q0")
                    pq1 = qps.tile([128, ST], F32, tag="q1")
                    pkv = qps.tile([128, ST], F32, tag="kv")
                    for kt in range(NDT):
                        st_flag, sp_flag = kt == 0, kt == NDT - 1
                        nc.tensor.matmul(pq0[:], r(wq_sb[:, kt, 0:128]), r(xts[kt][:]),
                                         start=st_flag, stop=sp_flag)
                        nc.tensor.matmul(pq1[:], r(wq_sb[:, kt, 128:256]), r(xts[kt][:]),
                                         start=st_flag, stop=sp_flag)
                        nc.tensor.matmul(pkv[:], r(wkv_sb[:, kt, :]), r(xts[kt][:]),
                                         start=st_flag, stop=sp_flag)
                    nc.scalar.activation(qT_raw[0][:, ssl], pq0[:],
                                         mybir.ActivationFunctionType.Identity,
                                         bias=bq_sb[:, 0:1])
                    nc.scalar.activation(qT_raw[1][:, ssl], pq1[:],
                                         mybir.ActivationFunctionType.Identity,
                                         bias=bq_sb[:, 1:2])
                    nc.scalar.activation(kv_raw[:, ssl], pkv[:],
                                         mybir.ActivationFunctionType.Identity,
                                         bias=bkv_sb[:, 0:1])

            # ---------------- Phase 1b: RoPE + v transpose ----------------
            with tc.tile_pool(name="rope", bufs=2) as rp, \
                 tc.tile_pool(name="vtps", bufs=2, space="PSUM") as vps:
                # v natural: transpose vT (kv_raw rows 0:64) into v_sb
                for j in range(NKT):
                    vt_ps = vps.tile([128, 64], F32)
                    nc.tensor.transpose(vt_ps[:], kv_raw[0:64, ts(j, 128)], ident[:])
                    nc.vector.tensor_copy(v_sb[:, j, 0:64], vt_ps[:])

                def rope(raw_ap, out_ap, rows):
                    # rows: list of 64-row head-block base offsets
                    sw = rp.tile([128, S], F32, tag="ropesw")
                    t1 = rp.tile([128, S], F32, tag="ropet1")
                    for b in rows:
                        nc.sync.dma_start(sw[b:b + 32, :], raw_ap[b + 32:b + 64, :])
                        nc.sync.dma_start(sw[b + 32:b + 64, :], raw_ap[b:b + 32, :])
                    lo, hi = rows[0], rows[-1] + 64
                    nc.vector.tensor_mul(t1[lo:hi, :], raw_ap[lo:hi, :], cc_sb[lo:hi, :])
                    nc.vector.tensor_mul(sw[lo:hi, :], sw[lo:hi, :], ss_sb[lo:hi, :])
                    nc.vector.tensor_add(out_ap[lo:hi, :], t1[lo:hi, :], sw[lo:hi, :])

                rope(qT_raw[0][:], qT[0][:], [0, 64])
                rope(qT_raw[1][:], qT[1][:], [0, 64])
                rope(kv_raw[:], kT[:], [64])
                nc.sync.dma_start(kT[0:64, :], kT[64:128, :])

            # ---------------- Phase 2: attention ----------------
            with tc.tile_pool(name="scps", bufs=3, space="PSUM") as scps, \
                 tc.tile_pool(name="avps", bufs=4, space="PSUM") as avps, \
                 tc.tile_pool(name="bcps", bufs=1, space="PSUM") as bcps, \
                 tc.tile_pool(name="ptp", bufs=6) as ptp, \
                 tc.tile_pool(name="nrm", bufs=2) as nrm, \
                 (tc.tile_pool(name="mskp", bufs=2) if use_mask else nullcontext()) as mskp:
                for t in range(NQT):
                    tsl = ts(t, QT)
                    n_k = 4 * (t + 1) if causal else NKT
                    aps = [avps.tile([65, QT], F32, tag="av", name=f"av{h}") for h in range(4)]
                    for j in range(n_k):
                        jsl = ts(j, 128)
                        if use_mask:
                            mt = mskp.tile([128, QT], F32, tag="mt")
                            nc.sync.dma_start(mt[:], maskt[jsl, tsl])
                        pts = []
                        for hp in range(2):
                            psA = scps.tile([128, QT], F32, tag="sc")
                            psB = scps.tile([128, QT], F32, tag="sc")
                            nc.tensor.matmul(psA[:], r(kT[0:64, jsl]), r(qT[hp][0:64, tsl]),
                                             start=True, stop=True)
                            nc.tensor.matmul(psB[:], r(kT[64:128, jsl]), r(qT[hp][64:128, tsl]),
                                             start=True, stop=True)
                            for hh, ps in ((0, psA), (1, psB)):
                                pt = ptp.tile([128, QT], F32R, tag="pt")
                                nc.scalar.activation(pt[:], ps[:],
                                                     mybir.ActivationFunctionType.Exp,
                                                     scale=0.125)
                                if causal and j >= 4 * t:
                                    v = j - 4 * t
                                    nc.vector.tensor_mul(pt[:], pt[:], tri_sb[:, v, :])
                                if use_mask:
                                    nc.vector.tensor_mul(pt[:], pt[:], mt[:])
                                pts.append((2 * hp + hh, pt))
                        for h, pt in pts:
                            nc.tensor.matmul(aps[h][:], r(v_sb[:, j, :]), r(pt[:]),
                                             start=(j == 0), stop=(j == n_k - 1))
                    # normalize: attn[hp] rows 0:64 = head 2hp, rows 64:128 = head 2hp+1
                    for h in range(4):
                        hp, hh = divmod(h, 2)
                        rc = nrm.tile([65, QT], F32, tag="rc")
                        nc.vector.reciprocal(rc[64:65, :], aps[h][64:65, :])
                        rcr = nrm.tile([65, QT], F32R, tag="rcr")
                        nc.scalar.activation(rcr[64:65, :], rc[64:65, :],
                                             mybir.ActivationFunctionType.Copy)
                        bc = bcps.tile([64, QT], F32, tag="bc")
                        nc.tensor.matmul(bc[:], r(ones65[64:65, 0:64]), rcr[64:65, :],
                                         start=True, stop=True)
                        bcs = nrm.tile([64, QT], F32, tag="bcs")
                        nc.any.tensor_copy(out=bcs[:], in_=bc[:])
                        if hh == 0:
                            nc.vector.tensor_mul(attn[hp][0:64, tsl], aps[h][0:64, :], bcs[:])
                        else:
                            tb = nrm.tile([64, QT], F32R, tag="tb")
                            nc.vector.tensor_mul(tb[:], aps[h][0:64, :], bcs[:])
                            nc.sync.dma_start(attn[hp][64:128, tsl], tb[:])

            # ---------------- Phase 3: output projection (partial) ----------------
            with tc.tile_pool(name="ops", bufs=8, space="PSUM") as ops, \
                 tc.tile_pool(name="osb", bufs=4) as osb:
                for ssub in range(S // 128):
                    psl = ts(ssub, 128)
                    pps = [ops.tile([128, 512], F32, tag="op", name=f"op{et}") for et in range(4)]
                    for ct in range(2):
                        for et in range(4):
                            nc.tensor.matmul(pps[et][:], r(attn[ct][:, psl]),
                                             r(wo_sb[:, ct, ts(et, 512)]),
                                             start=(ct == 0), stop=(ct == 1))
                    for et in range(4):
                        ot = osb.tile([128, 512], F32, tag="ot")
                        nc.any.tensor_copy(out=ot[:], in_=pps[et][:])
                        nc.sync.dma_start(partial[psl, ts(et, 512)], ot[:])

    nc.compile()
    return nc


_CACHE = {}
TRACE = False
LAST_EXEC_NS = None


def _get(causal, use_mask):
    key = (causal, use_mask)
    if key not in _CACHE:
        _CACHE[key] = _build(causal, use_mask)
    return _CACHE[key]


def _perm_eo(w):
    # de-interleave channel pairs per 64-col head block: [evens, odds]
    cols = np.concatenate([np.arange(0, 64, 2), np.arange(1, 64, 2)])
    return w[..., cols]


def kernel(**inputs):
    x = np.asarray(inputs["x"], dtype=np.float32)
    fc = np.asarray(inputs["freqs_cos"], dtype=np.float32)
    fs = np.asarray(inputs["freqs_sin"], dtype=np.float32)
    mask = np.asarray(inputs["mask"])
    Wq = np.asarray(inputs["Wq"], dtype=np.float32)
    bq = np.asarray(inputs["bq"], dtype=np.float32)
    Wk = np.asarray(inputs["Wk"], dtype=np.float32)
    bk = np.asarray(inputs["bk"], dtype=np.float32)
    Wv = np.asarray(inputs["Wv"], dtype=np.float32)
    bv = np.asarray(inputs["bv"], dtype=np.float32)
    Wo = np.asarray(inputs["Wo"], dtype=np.float32)
    bo = np.asarray(inputs["bo"], dtype=np.float32)

    m2 = mask.reshape(S, S)
    if (m2 == 1).all():
        causal, use_mask = False, False
    elif np.array_equal(m2 != 0, np.tril(np.ones((S, S), dtype=bool))):
        causal, use_mask = True, False
    else:
        causal, use_mask = False, True
    nc = _get(causal, use_mask)

    xT = np.ascontiguousarray(x[0].T)
    cosT = np.ascontiguousarray(fc.T)  # (32, S)
    sinT = np.ascontiguousarray(fs.T)
    cc = np.tile(cosT, (4, 1))
    ssgn = np.concatenate([-sinT, sinT, -sinT, sinT], axis=0)
    kl = np.arange(128)[:, None]
    qq = np.arange(512)[None, :]
    tri = np.stack([(qq >= 128 * v + kl) for v in range(4)]).astype(np.float32)
    tri = np.ascontiguousarray(tri.reshape(512, 512))

    Wq_h = Wq.reshape(DIM, H, D)
    bq_h = bq.reshape(H, D)
    Wk_h = Wk.reshape(DIM, KVH, D)
    bk_h = bk.reshape(KVH, D)

    in_maps = []
    for c in range(NCORES):
        hs = slice(HQ * c, HQ * (c + 1))
        wq_c = _perm_eo(Wq_h[:, hs, :].transpose(0, 1, 2)).reshape(DIM, CQ)
        bq_c = _perm_eo(bq_h[hs, :]).reshape(CQ)
        wk_c = _perm_eo(Wk_h[:, c, :])
        bk_c = _perm_eo(bk_h[c, :])
        wv_c = Wv[:, 64 * c:64 * (c + 1)]
        bv_c = bv[64 * c:64 * (c + 1)]
        wkv_c = np.ascontiguousarray(np.concatenate([wv_c, wk_c], axis=1))
        bkv_c = np.concatenate([bv_c, bk_c])
        wo_c = np.ascontiguousarray(Wo[CQ * c:CQ * (c + 1), :])
        im = {
            "xT": xT, "wq": np.ascontiguousarray(wq_c), "wkv": wkv_c,
            "wo": wo_c, "bq": np.ascontiguousarray(bq_c),
            "bkv": np.ascontiguousarray(bkv_c), "cc": np.ascontiguousarray(cc),
            "ssgn": np.ascontiguousarray(ssgn), "trimask": tri,
            "ones": np.ones(128, dtype=np.float32),
        }
        if use_mask:
            im["maskt"] = np.ascontiguousarray(m2.T.astype(np.float32))
        in_maps.append(im)

    global LAST_EXEC_NS
    res = run_bass_kernel_spmd(nc, in_maps, core_ids=list(range(NCORES)), trace=TRACE)
    LAST_EXEC_NS = res.exec_time_ns
    out = np.zeros((S, DIM), dtype=np.float32)
    for rr in res.results:
        out += rr["partial"]
    out += bo
    return out.reshape(1, S, DIM)


# revision 16
# speedup vs baseline: 1.0769x; 1.0769x over previous
"""LLaMA causal self-attention, 8-way head-tensor-parallel Trainium2 Bass kernel.

Sharding: each of 8 cores computes 4 query heads + its 1 KV head-group
(Wq/Wk/Wv column-sharded), plus a row-shard of Wo producing a partial
(S, DIM) output; partials are summed on the host (the all-reduce of the
row-sharded Wo matmul).

Device layout notes:
  - x is passed pre-transposed (xT, [DIM, S]) so projection matmuls contract
    over partitions without on-device transposes.
  - q/k channel order is host-permuted per head to [evens, odds] so RoPE pair
    mixing becomes 32-row block ops (partition-aligned; the pair-swap operand
    is built with SBUF->SBUF DMAs).
  - scores are computed transposed (scoresT[k, q]) so exp(P) feeds P@V as the
    moving operand with k on partitions; a ones-column appended to v yields
    the softmax row-sums as psum row 64.
  - softmax normalization: reciprocal of sums row, rank-1 (K=1) PE matmul to
    broadcast it across 64 partitions, then one DVE multiply.
"""

import numpy as np

import concourse.bass as bass
import concourse.mybir as mybir
import concourse.tile as tile
from contextlib import nullcontext
from concourse import bacc
from concourse.bass import ts, ds
from concourse.bass_utils import run_bass_kernel_spmd
from concourse.masks import make_identity

F32 = mybir.dt.float32
F32R = mybir.dt.float32r

S = 2048
DIM = 2048
H = 32
KVH = 8
D = 64
NCORES = 8
HQ = H // NCORES          # 4 q heads per core
CQ = HQ * D               # 256 q cols per core
ST = 256                  # s-tile width in QKV projection phase
QT = 512                  # q-tile width in attention
NKT = S // 128            # 16 key tiles
NDT = DIM // 128          # 16 contraction tiles for projections
NST = S // ST             # 8 projection s-tiles
NQT = S // QT             # 4 attention q-tiles


def r(ap):
    return ap.bitcast(F32R)


def _build(causal: bool, use_mask: bool):
    nc = bacc.Bacc("TRN2", target_bir_lowering=False, debug=False,
                   num_devices=NCORES, name="llama_attn")
    xT = nc.dram_tensor("xT", [DIM, S], F32R, kind="ExternalInput")
    wq = nc.dram_tensor("wq", [DIM, CQ], F32R, kind="ExternalInput")
    wkv = nc.dram_tensor("wkv", [DIM, 128], F32R, kind="ExternalInput")
    wo = nc.dram_tensor("wo", [CQ, DIM], F32R, kind="ExternalInput")
    bqd = nc.dram_tensor("bq", [CQ], F32, kind="ExternalInput")
    bkvd = nc.dram_tensor("bkv", [128], F32, kind="ExternalInput")
    ccd = nc.dram_tensor("cc", [128, S], F32, kind="ExternalInput")
    ssd = nc.dram_tensor("ssgn", [128, S], F32, kind="ExternalInput")
    trid = nc.dram_tensor("trimask", [512, 512], F32, kind="ExternalInput")
    onesd = nc.dram_tensor("ones", [128], F32R, kind="ExternalInput")
    if use_mask:
        maskt = nc.dram_tensor("maskt", [S, S], F32, kind="ExternalInput")
    partial = nc.dram_tensor("partial", [S, DIM], F32, kind="ExternalOutput")

    with tile.TileContext(nc) as tc:
        with tc.tile_pool(name="persist", bufs=1) as pp:
            wq_sb = pp.tile([128, NDT, CQ], F32R)
            nc.sync.dma_start(wq_sb[:], wq.ap().rearrange("(a p) c -> p a c", p=128))
            wkv_sb = pp.tile([128, NDT, 128], F32R)
            nc.sync.dma_start(wkv_sb[:], wkv.ap().rearrange("(a p) c -> p a c", p=128))
            wo_sb = pp.tile([128, 2, DIM], F32R)
            nc.sync.dma_start(wo_sb[:], wo.ap().rearrange("(a p) e -> p a e", p=128))
            cc_sb = pp.tile([128, S], F32)
            nc.sync.dma_start(cc_sb[:], ccd[:])
            ss_sb = pp.tile([128, S], F32)
            nc.sync.dma_start(ss_sb[:], ssd[:])
            bq_sb = pp.tile([128, 2], F32)
            nc.sync.dma_start(bq_sb[:], bqd.ap().rearrange("(t p) -> p t", p=128))
            bkv_sb = pp.tile([128, 1], F32)
            nc.sync.dma_start(bkv_sb[:], bkvd.ap()[:, None])
            tri_sb = pp.tile([128, 4, 512], F32)
            ones65 = pp.tile([65, 128], F32R, name="ones65")
            nc.gpsimd.dma_start(ones65[64:65, :], onesd.ap()[None, :])
            nc.sync.dma_start(tri_sb[:], trid.ap().rearrange("(v p) q -> p v q", p=128))
            ident = pp.tile([64, 64], F32)
            make_identity(nc, ident[:])

            # persistent activations
            qT_raw = [pp.tile([128, S], F32, tag=f"qtraw{i}", name=f"qtraw{i}") for i in range(2)]
            kv_raw = pp.tile([128, S], F32)
            qT = [pp.tile([128, S], F32R, tag=f"qt{i}", name=f"qt{i}") for i in range(2)]
            kT = pp.tile([128, S], F32R)          # rows 0:64 and 64:128 both = roped k
            v_sb = pp.tile([128, NKT, 65], F32R)  # [kpos, ktile, ch + ones]
            nc.gpsimd.dma_start(v_sb[:, :, 64:65],
                                onesd.ap()[:, None, None].to_broadcast((128, NKT, 1)))
            attn = [[pp.tile([128, QT], F32R, tag=f"attn{c}_{t}", name=f"attn{c}_{t}")
                     for t in range(NQT)] for c in range(2)]

            # ---------------- Phase 1a: QKV projections ----------------
            with tc.tile_pool(name="xstream", bufs=2) as xp, \
                 tc.tile_pool(name="qkvps", bufs=2, space="PSUM") as qps:
                for st in range(NST):
                    ssl = ts(st, ST)
                    xts = []
                    for kt in range(NDT):
                        xt = xp.tile([128, ST], F32R, tag=f"xt{kt}")
                        nc.sync.dma_start(xt[:], xT[ds(kt * 128, 128), ssl])
                        xts.append(xt)
                    pq0 = qps.tile([128, ST], F32, tag="q0")
                    pq1 = qps.tile([128, ST], F32, tag="q1")
                    pkv = qps.tile([128, ST], F32, tag="kv")
                    for kt in range(NDT):
                        st_flag, sp_flag = kt == 0, kt == NDT - 1
                        nc.tensor.matmul(pq0[:], r(wq_sb[:, kt, 0:128]), r(xts[kt][:]),
                                         start=st_flag, stop=sp_flag)
                        nc.tensor.matmul(pq1[:], r(wq_sb[:, kt, 128:256]), r(xts[kt][:]),
                                         start=st_flag, stop=sp_flag)
                        nc.tensor.matmul(pkv[:], r(wkv_sb[:, kt, :]), r(xts[kt][:]),
                                         start=st_flag, stop=sp_flag)
                    nc.scalar.activation(qT_raw[0][:, ssl], pq0[:],
                                         mybir.ActivationFunctionType.Identity,
                                         bias=bq_sb[:, 0:1])
                    nc.scalar.activation(qT_raw[1][:, ssl], pq1[:],
                                         mybir.ActivationFunctionType.Identity,
                                         bias=bq_sb[:, 1:2])
                    nc.scalar.activation(kv_raw[:, ssl], pkv[:],
                                         mybir.ActivationFunctionType.Identity,
                                         bias=bkv_sb[:, 0:1])

            # ---------------- Phase 1b: RoPE + v transpose ----------------
            with tc.tile_pool(name="rope", bufs=2) as rp, \
                 tc.tile_pool(name="vtps", bufs=2, space="PSUM") as vps:
                # v natural: transpose vT (kv_raw rows 0:64) into v_sb
                for j in range(NKT):
                    vt_ps = vps.tile([128, 64], F32)
                    nc.tensor.transpose(vt_ps[:], kv_raw[0:64, ts(j, 128)], ident[:])
                    nc.vector.tensor_copy(v_sb[:, j, 0:64], vt_ps[:])

                def rope(raw_ap, out_ap, rows):
                    # rows: list of 64-row head-block base offsets
                    sw = rp.tile([128, S], F32, tag="ropesw")
                    t1 = rp.tile([128, S], F32, tag="ropet1")
                    for b in rows:
                        nc.sync.dma_start(sw[b:b + 32, :], raw_ap[b + 32:b + 64, :])
                        nc.sync.dma_start(sw[b + 32:b + 64, :], raw_ap[b:b + 32, :])
                    lo, hi = rows[0], rows[-1] + 64
                    nc.vector.tensor_mul(t1[lo:hi, :], raw_ap[lo:hi, :], cc_sb[lo:hi, :])
                    nc.vector.tensor_mul(sw[lo:hi, :], sw[lo:hi, :], ss_sb[lo:hi, :])
                    nc.vector.tensor_add(out_ap[lo:hi, :], t1[lo:hi, :], sw[lo:hi, :])

                rope(qT_raw[0][:], qT[0][:], [0, 64])
                rope(qT_raw[1][:], qT[1][:], [0, 64])
                rope(kv_raw[:], kT[:], [64])
                nc.sync.dma_start(kT[0:64, :], kT[64:128, :])

            # ------- Phase 2+3 fused: attention with interleaved out-proj -------
            with tc.tile_pool(name="scps", bufs=3, space="PSUM") as scps, \
                 tc.tile_pool(name="avps", bufs=2, space="PSUM") as avps, \
                 tc.tile_pool(name="bcps", bufs=1, space="PSUM") as bcps, \
                 tc.tile_pool(name="opsp", bufs=2, space="PSUM") as opsp, \
                 tc.tile_pool(name="ptp", bufs=4) as ptp, \
                 tc.tile_pool(name="nrm", bufs=3) as nrm, \
                 tc.tile_pool(name="osb", bufs=4) as osb, \
                 (tc.tile_pool(name="mskp", bufs=2) if use_mask else nullcontext()) as mskp:
                for t in range(NQT):
                    tsl = ts(t, QT)
                    n_k = 4 * (t + 1) if causal else NKT
                    for hp in range(2):
                        aps2 = [avps.tile([65, QT], F32, tag="av", name=f"av{t}_{hp}_{hh}")
                                for hh in range(2)]
                        for j in range(n_k):
                            jsl = ts(j, 128)
                            if use_mask:
                                mt = mskp.tile([128, QT], F32, tag="mt")
                                nc.sync.dma_start(mt[:], maskt[jsl, tsl])
                            psA = scps.tile([128, QT], F32, tag="sc", name="psA")
                            psB = scps.tile([128, QT], F32, tag="sc", name="psB")
                            nc.tensor.matmul(psA[:], kT[0:64, jsl], qT[hp][0:64, tsl],
                                             start=True, stop=True)
                            nc.tensor.matmul(psB[:], kT[64:128, jsl], qT[hp][64:128, tsl],
                                             start=True, stop=True)
                            for hh, ps in ((0, psA), (1, psB)):
                                pt = ptp.tile([128, QT], F32R, tag="pt")
                                nc.scalar.activation(pt[:], ps[:],
                                                     mybir.ActivationFunctionType.Exp,
                                                     scale=0.125)
                                if causal and j >= 4 * t:
                                    v = j - 4 * t
                                    nc.vector.tensor_mul(pt[:], pt[:], tri_sb[:, v, :])
                                if use_mask:
                                    nc.vector.tensor_mul(pt[:], pt[:], mt[:])
                                nc.tensor.matmul(aps2[hh][:], v_sb[:, j, :], pt[:],
                                                 start=(j == 0), stop=(j == n_k - 1))
                        for hh in range(2):
                            h = 2 * hp + hh
                            avsb = nrm.tile([65, QT], F32, tag="avsb")
                            nc.scalar.activation(avsb[:], aps2[hh][:],
                                                 mybir.ActivationFunctionType.Identity)
                            rc = nrm.tile([65, QT], F32, tag="rc")
                            nc.vector.reciprocal(rc[64:65, :], avsb[64:65, :])
                            rcr = nrm.tile([65, QT], F32R, tag="rcr")
                            nc.scalar.activation(rcr[64:65, :], rc[64:65, :],
                                                 mybir.ActivationFunctionType.Copy)
                            bc = bcps.tile([64, QT], F32, tag="bc")
                            nc.tensor.matmul(bc[:], r(ones65[64:65, 0:64]), rcr[64:65, :],
                                             start=True, stop=True)
                            bcs = nrm.tile([64, QT], F32, tag="bcs")
                            nc.vector.tensor_copy(bcs[:], bc[:])
                            if hh == 0:
                                nc.vector.tensor_mul(attn[hp][t][0:64, :],
                                                     avsb[0:64, :], bcs[:])
                            else:
                                tb = nrm.tile([64, QT], F32R, tag="tb")
                                nc.vector.tensor_mul(tb[:], avsb[0:64, :], bcs[:])
                                nc.sync.dma_start(attn[hp][t][64:128, :], tb[:])
                    # out-projection rows for this t
                    for sl in range(4):
                        ssub = 4 * t + sl
                        for et in range(4):
                            pps = opsp.tile([128, 512], F32, tag="op")
                            for ct in range(2):
                                nc.tensor.matmul(pps[:], attn[ct][t][:, ts(sl, 128)],
                                                 wo_sb[:, ct, ts(et, 512)],
                                                 start=(ct == 0), stop=(ct == 1))
                            ot = osb.tile([128, 512], F32, tag="ot")
                            nc.vector.tensor_copy(ot[:], pps[:])
                            nc.sync.dma_start(partial[ts(ssub, 128), ts(et, 512)], ot[:])

    nc.compile()
    return nc


_CACHE = {}
TRACE = False
LAST_EXEC_NS = None
LAST_RES = None


def _get(causal, use_mask):
    key = (causal, use_mask)
    if key not in _CACHE:
        _CACHE[key] = _build(causal, use_mask)
    return _CACHE[key]


def _perm_eo(w):
    # de-interleave channel pairs per 64-col head block: [evens, odds]
    cols = np.concatenate([np.arange(0, 64, 2), np.arange(1, 64, 2)])
    return w[..., cols]


def kernel(**inputs):
    x = np.asarray(inputs["x"], dtype=np.float32)
    fc = np.asarray(inputs["freqs_cos"], dtype=np.float32)
    fs = np.asarray(inputs["freqs_sin"], dtype=np.float32)
    mask = np.asarray(inputs["mask"])
    Wq = np.asarray(inputs["Wq"], dtype=np.float32)
    bq = np.asarray(inputs["bq"], dtype=np.float32)
    Wk = np.asarray(inputs["Wk"], dtype=np.float32)
    bk = np.asarray(inputs["bk"], dtype=np.float32)
    Wv = np.asarray(inputs["Wv"], dtype=np.float32)
    bv = np.asarray(inputs["bv"], dtype=np.float32)
    Wo = np.asarray(inputs["Wo"], dtype=np.float32)
    bo = np.asarray(inputs["bo"], dtype=np.float32)

    m2 = mask.reshape(S, S)
    if (m2 == 1).all():
        causal, use_mask = False, False
    elif np.array_equal(m2 != 0, np.tril(np.ones((S, S), dtype=bool))):
        causal, use_mask = True, False
    else:
        causal, use_mask = False, True
    nc = _get(causal, use_mask)

    xT = np.ascontiguousarray(x[0].T)
    cosT = np.ascontiguousarray(fc.T)  # (32, S)
    sinT = np.ascontiguousarray(fs.T)
    cc = np.tile(cosT, (4, 1))
    ssgn = np.concatenate([-sinT, sinT, -sinT, sinT], axis=0)
    kl = np.arange(128)[:, None]
    qq = np.arange(512)[None, :]
    tri = np.stack([(qq >= 128 * v + kl) for v in range(4)]).astype(np.float32)
    tri = np.ascontiguousarray(tri.reshape(512, 512))

    Wq_h = Wq.reshape(DIM, H, D)
    bq_h = bq.reshape(H, D)
    Wk_h = Wk.reshape(DIM, KVH, D)
    bk_h = bk.reshape(KVH, D)

    in_maps = []
    for c in range(NCORES):
        hs = slice(HQ * c, HQ * (c + 1))
        wq_c = _perm_eo(Wq_h[:, hs, :].transpose(0, 1, 2)).reshape(DIM, CQ)
        bq_c = _perm_eo(bq_h[hs, :]).reshape(CQ)
        wk_c = _perm_eo(Wk_h[:, c, :])
        bk_c = _perm_eo(bk_h[c, :])
        wv_c = Wv[:, 64 * c:64 * (c + 1)]
        bv_c = bv[64 * c:64 * (c + 1)]
        wkv_c = np.ascontiguousarray(np.concatenate([wv_c, wk_c], axis=1))
        bkv_c = np.concatenate([bv_c, bk_c])
        wo_c = np.ascontiguousarray(Wo[CQ * c:CQ * (c + 1), :])
        im = {
            "xT": xT, "wq": np.ascontiguousarray(wq_c), "wkv": wkv_c,
            "wo": wo_c, "bq": np.ascontiguousarray(bq_c),
            "bkv": np.ascontiguousarray(bkv_c), "cc": np.ascontiguousarray(cc),
            "ssgn": np.ascontiguousarray(ssgn), "trimask": tri,
            "ones": np.ones(128, dtype=np.float32),
        }
        if use_mask:
            im["maskt"] = np.ascontiguousarray(m2.T.astype(np.float32))
        in_maps.append(im)

    global LAST_EXEC_NS, LAST_RES
    res = run_bass_kernel_spmd(nc, in_maps, core_ids=list(range(NCORES)), trace=TRACE)
    LAST_EXEC_NS = res.exec_time_ns
    LAST_RES = res
    out = np.zeros((S, DIM), dtype=np.float32)
    for rr in res.results:
        out += rr["partial"]
    out += bo
    return out.reshape(1, S, DIM)


# revision 17
# speedup vs baseline: 1.1494x; 1.0674x over previous
"""LLaMA causal self-attention, 8-way head-tensor-parallel Trainium2 Bass kernel.

Sharding: each of 8 cores computes 4 query heads + its 1 KV head-group
(Wq/Wk/Wv column-sharded), plus a row-shard of Wo producing a partial
(S, DIM) output; partials are summed on the host (the all-reduce of the
row-sharded Wo matmul).

Device layout notes:
  - x is passed pre-transposed (xT, [DIM, S]) so projection matmuls contract
    over partitions without on-device transposes.
  - q/k channel order is host-permuted per head to [evens, odds] so RoPE pair
    mixing becomes 32-row block ops (partition-aligned; the pair-swap operand
    is built with SBUF->SBUF DMAs).
  - scores are computed transposed (scoresT[k, q]) so exp(P) feeds P@V as the
    moving operand with k on partitions; a ones-column appended to v yields
    the softmax row-sums as psum row 64.
  - softmax normalization: reciprocal of sums row, rank-1 (K=1) PE matmul to
    broadcast it across 64 partitions, then one DVE multiply.
"""

import numpy as np

import concourse.bass as bass
import concourse.mybir as mybir
import concourse.tile as tile
from contextlib import nullcontext
from concourse import bacc
from concourse.bass import ts, ds
from concourse.bass_utils import run_bass_kernel_spmd
from concourse.masks import make_identity

F32 = mybir.dt.float32
F32R = mybir.dt.float32r

S = 2048
DIM = 2048
H = 32
KVH = 8
D = 64
NCORES = 8
HQ = H // NCORES          # 4 q heads per core
CQ = HQ * D               # 256 q cols per core
ST = 256                  # s-tile width in QKV projection phase
QT = 512                  # q-tile width in attention
NKT = S // 128            # 16 key tiles
NDT = DIM // 128          # 16 contraction tiles for projections
NST = S // ST             # 8 projection s-tiles
NQT = S // QT             # 4 attention q-tiles


def r(ap):
    return ap.bitcast(F32R)


def _build(causal: bool, use_mask: bool):
    nc = bacc.Bacc("TRN2", target_bir_lowering=False, debug=False,
                   num_devices=NCORES, name="llama_attn")
    xT = nc.dram_tensor("xT", [DIM, S], F32R, kind="ExternalInput")
    wq = nc.dram_tensor("wq", [DIM, CQ], F32R, kind="ExternalInput")
    wkv = nc.dram_tensor("wkv", [DIM, 128], F32R, kind="ExternalInput")
    wo = nc.dram_tensor("wo", [CQ, DIM], F32R, kind="ExternalInput")
    bqd = nc.dram_tensor("bq", [CQ], F32, kind="ExternalInput")
    bkvd = nc.dram_tensor("bkv", [128], F32, kind="ExternalInput")
    ccd = nc.dram_tensor("cc", [128, S], F32, kind="ExternalInput")
    ssd = nc.dram_tensor("ssgn", [128, S], F32, kind="ExternalInput")
    trid = nc.dram_tensor("trimask", [512, 512], F32, kind="ExternalInput")
    onesd = nc.dram_tensor("ones", [128], F32R, kind="ExternalInput")
    if use_mask:
        maskt = nc.dram_tensor("maskt", [S, S], F32, kind="ExternalInput")
    partial = nc.dram_tensor("partial", [S, DIM], F32, kind="ExternalOutput")

    with tile.TileContext(nc) as tc:
        with tc.tile_pool(name="persist", bufs=1) as pp:
            wq_sb = pp.tile([128, NDT, CQ], F32R)
            nc.sync.dma_start(wq_sb[:], wq.ap().rearrange("(a p) c -> p a c", p=128))
            wkv_sb = pp.tile([128, NDT, 128], F32R)
            nc.sync.dma_start(wkv_sb[:], wkv.ap().rearrange("(a p) c -> p a c", p=128))
            wo_sb = pp.tile([128, 2, DIM], F32R)
            nc.sync.dma_start(wo_sb[:], wo.ap().rearrange("(a p) e -> p a e", p=128))
            cc_sb = pp.tile([128, S], F32)
            nc.sync.dma_start(cc_sb[:], ccd[:])
            ss_sb = pp.tile([128, S], F32)
            nc.sync.dma_start(ss_sb[:], ssd[:])
            bq_sb = pp.tile([128, 2], F32)
            nc.sync.dma_start(bq_sb[:], bqd.ap().rearrange("(t p) -> p t", p=128))
            bkv_sb = pp.tile([128, 1], F32)
            nc.sync.dma_start(bkv_sb[:], bkvd.ap()[:, None])
            tri_sb = pp.tile([128, 4, 512], F32)
            ones65 = pp.tile([65, 128], F32R, name="ones65")
            nc.gpsimd.dma_start(ones65[64:65, :], onesd.ap()[None, :])
            nc.sync.dma_start(tri_sb[:], trid.ap().rearrange("(v p) q -> p v q", p=128))
            ident = pp.tile([64, 64], F32)
            make_identity(nc, ident[:])

            # persistent activations
            qT_raw = [pp.tile([128, S], F32, tag=f"qtraw{i}", name=f"qtraw{i}") for i in range(2)]
            kv_raw = pp.tile([128, S], F32)
            qT = [pp.tile([128, S], F32R, tag=f"qt{i}", name=f"qt{i}") for i in range(2)]
            kT = pp.tile([128, S], F32R)          # rows 0:64 and 64:128 both = roped k
            v_sb = pp.tile([128, NKT, 65], F32R)  # [kpos, ktile, ch + ones]
            nc.gpsimd.dma_start(v_sb[:, :, 64:65],
                                onesd.ap()[:, None, None].to_broadcast((128, NKT, 1)))
            attn = [[pp.tile([128, QT], F32R, tag=f"attn{c}_{t}", name=f"attn{c}_{t}")
                     for t in range(NQT)] for c in range(2)]

            # ---------------- Phase 1a: QKV projections ----------------
            with tc.tile_pool(name="xstream", bufs=2) as xp, \
                 tc.tile_pool(name="qkvps", bufs=2, space="PSUM") as qps:
                for st in range(NST):
                    ssl = ts(st, ST)
                    xts = []
                    for kt in range(NDT):
                        xt = xp.tile([128, ST], F32R, tag=f"xt{kt}")
                        nc.sync.dma_start(xt[:], xT[ds(kt * 128, 128), ssl])
                        xts.append(xt)
                    pq0 = qps.tile([128, ST], F32, tag="q0")
                    pq1 = qps.tile([128, ST], F32, tag="q1")
                    pkv = qps.tile([128, ST], F32, tag="kv")
                    for kt in range(NDT):
                        st_flag, sp_flag = kt == 0, kt == NDT - 1
                        nc.tensor.matmul(pq0[:], r(wq_sb[:, kt, 0:128]), r(xts[kt][:]),
                                         start=st_flag, stop=sp_flag)
                        nc.tensor.matmul(pq1[:], r(wq_sb[:, kt, 128:256]), r(xts[kt][:]),
                                         start=st_flag, stop=sp_flag)
                        nc.tensor.matmul(pkv[:], r(wkv_sb[:, kt, :]), r(xts[kt][:]),
                                         start=st_flag, stop=sp_flag)
                    nc.scalar.activation(qT_raw[0][:, ssl], pq0[:],
                                         mybir.ActivationFunctionType.Identity,
                                         bias=bq_sb[:, 0:1])
                    nc.scalar.activation(qT_raw[1][:, ssl], pq1[:],
                                         mybir.ActivationFunctionType.Identity,
                                         bias=bq_sb[:, 1:2])
                    nc.scalar.activation(kv_raw[:, ssl], pkv[:],
                                         mybir.ActivationFunctionType.Identity,
                                         bias=bkv_sb[:, 0:1])

            # ---------------- Phase 1b: RoPE + v transpose ----------------
            with tc.tile_pool(name="rope", bufs=2) as rp, \
                 tc.tile_pool(name="vtps", bufs=2, space="PSUM") as vps:
                # v natural: transpose vT (kv_raw rows 0:64) into v_sb
                for j in range(NKT):
                    vt_ps = vps.tile([128, 64], F32)
                    nc.tensor.transpose(vt_ps[:], kv_raw[0:64, ts(j, 128)], ident[:])
                    nc.vector.tensor_copy(v_sb[:, j, 0:64], vt_ps[:])

                def rope(raw_ap, out_ap, rows):
                    # rows: list of 64-row head-block base offsets
                    sw = rp.tile([128, S], F32, tag="ropesw")
                    t1 = rp.tile([128, S], F32, tag="ropet1")
                    for b in rows:
                        nc.sync.dma_start(sw[b:b + 32, :], raw_ap[b + 32:b + 64, :])
                        nc.sync.dma_start(sw[b + 32:b + 64, :], raw_ap[b:b + 32, :])
                    lo, hi = rows[0], rows[-1] + 64
                    nc.vector.tensor_mul(t1[lo:hi, :], raw_ap[lo:hi, :], cc_sb[lo:hi, :])
                    nc.vector.tensor_mul(sw[lo:hi, :], sw[lo:hi, :], ss_sb[lo:hi, :])
                    nc.vector.tensor_add(out_ap[lo:hi, :], t1[lo:hi, :], sw[lo:hi, :])

                rope(qT_raw[0][:], qT[0][:], [0, 64])
                rope(qT_raw[1][:], qT[1][:], [0, 64])
                rope(kv_raw[:], kT[:], [64])
                nc.sync.dma_start(kT[0:64, :], kT[64:128, :])

            # ------- Phase 2+3 fused: attention with interleaved out-proj -------
            with tc.tile_pool(name="scps", bufs=3, space="PSUM") as scps, \
                 tc.tile_pool(name="avps", bufs=2, space="PSUM") as avps, \
                 tc.tile_pool(name="bcps", bufs=1, space="PSUM") as bcps, \
                 tc.tile_pool(name="opsp", bufs=2, space="PSUM") as opsp, \
                 tc.tile_pool(name="ptp", bufs=4) as ptp, \
                 tc.tile_pool(name="nrm", bufs=3) as nrm, \
                 tc.tile_pool(name="osb", bufs=4) as osb, \
                 (tc.tile_pool(name="mskp", bufs=2) if use_mask else nullcontext()) as mskp:
                for t in range(NQT):
                    tsl = ts(t, QT)
                    n_k = 4 * (t + 1) if causal else NKT
                    for hp in range(2):
                        aps2 = [avps.tile([65, QT], F32, tag="av", name=f"av{t}_{hp}_{hh}")
                                for hh in range(2)]
                        for j in range(n_k):
                            jsl = ts(j, 128)
                            if use_mask:
                                mt = mskp.tile([128, QT], F32, tag="mt")
                                nc.sync.dma_start(mt[:], maskt[jsl, tsl])
                            psA = scps.tile([128, QT], F32, tag="sc", name="psA")
                            psB = scps.tile([128, QT], F32, tag="sc", name="psB")
                            nc.tensor.matmul(psA[:], kT[0:64, jsl], qT[hp][0:64, tsl],
                                             start=True, stop=True)
                            nc.tensor.matmul(psB[:], kT[64:128, jsl], qT[hp][64:128, tsl],
                                             start=True, stop=True)
                            for hh, ps in ((0, psA), (1, psB)):
                                pt = ptp.tile([128, QT], F32R, tag="pt")
                                nc.scalar.activation(pt[:], ps[:],
                                                     mybir.ActivationFunctionType.Exp,
                                                     scale=0.125)
                                if causal and j >= 4 * t:
                                    v = j - 4 * t
                                    nc.vector.tensor_mul(pt[:], pt[:], tri_sb[:, v, :])
                                if use_mask:
                                    nc.vector.tensor_mul(pt[:], pt[:], mt[:])
                                nc.tensor.matmul(aps2[hh][:], v_sb[:, j, :], pt[:],
                                                 start=(j == 0), stop=(j == n_k - 1))
                        for hh in range(2):
                            h = 2 * hp + hh
                            avsb = nrm.tile([65, QT], F32, tag="avsb")
                            nc.scalar.activation(avsb[:], aps2[hh][:],
                                                 mybir.ActivationFunctionType.Identity)
                            rc = nrm.tile([65, QT], F32, tag="rc")
                            nc.vector.reciprocal_approx_fast(rc[:, :], avsb[:, :])
                            rcr = nrm.tile([65, QT], F32R, tag="rcr")
                            nc.scalar.activation(rcr[64:65, :], rc[64:65, :],
                                                 mybir.ActivationFunctionType.Copy)
                            bc = bcps.tile([64, QT], F32, tag="bc")
                            nc.tensor.matmul(bc[:], r(ones65[64:65, 0:64]), rcr[64:65, :],
                                             start=True, stop=True)
                            bcs = nrm.tile([64, QT], F32, tag="bcs")
                            nc.vector.tensor_copy(bcs[:], bc[:])
                            if hh == 0:
                                nc.vector.tensor_mul(attn[hp][t][0:64, :],
                                                     avsb[0:64, :], bcs[:])
                            else:
                                tb = nrm.tile([64, QT], F32R, tag="tb")
                                nc.vector.tensor_mul(tb[:], avsb[0:64, :], bcs[:])
                                nc.sync.dma_start(attn[hp][t][64:128, :], tb[:])
                    # out-projection rows for this t
                    for sl in range(4):
                        ssub = 4 * t + sl
                        for et in range(4):
                            pps = opsp.tile([128, 512], F32, tag="op")
                            for ct in range(2):
                                nc.tensor.matmul(pps[:], attn[ct][t][:, ts(sl, 128)],
                                                 wo_sb[:, ct, ts(et, 512)],
                                                 start=(ct == 0), stop=(ct == 1))
                            ot = osb.tile([128, 512], F32, tag="ot")
                            nc.vector.tensor_copy(ot[:], pps[:])
                            nc.sync.dma_start(partial[ts(ssub, 128), ts(et, 512)], ot[:])

    nc.compile()
    return nc


_CACHE = {}
TRACE = False
LAST_EXEC_NS = None
LAST_RES = None


def _get(causal, use_mask):
    key = (causal, use_mask)
    if key not in _CACHE:
        _CACHE[key] = _build(causal, use_mask)
    return _CACHE[key]


def _perm_eo(w):
    # de-interleave channel pairs per 64-col head block: [evens, odds]
    cols = np.concatenate([np.arange(0, 64, 2), np.arange(1, 64, 2)])
    return w[..., cols]


def kernel(**inputs):
    x = np.asarray(inputs["x"], dtype=np.float32)
    fc = np.asarray(inputs["freqs_cos"], dtype=np.float32)
    fs = np.asarray(inputs["freqs_sin"], dtype=np.float32)
    mask = np.asarray(inputs["mask"])
    Wq = np.asarray(inputs["Wq"], dtype=np.float32)
    bq = np.asarray(inputs["bq"], dtype=np.float32)
    Wk = np.asarray(inputs["Wk"], dtype=np.float32)
    bk = np.asarray(inputs["bk"], dtype=np.float32)
    Wv = np.asarray(inputs["Wv"], dtype=np.float32)
    bv = np.asarray(inputs["bv"], dtype=np.float32)
    Wo = np.asarray(inputs["Wo"], dtype=np.float32)
    bo = np.asarray(inputs["bo"], dtype=np.float32)

    m2 = mask.reshape(S, S)
    if (m2 == 1).all():
        causal, use_mask = False, False
    elif np.array_equal(m2 != 0, np.tril(np.ones((S, S), dtype=bool))):
        causal, use_mask = True, False
    else:
        causal, use_mask = False, True
    nc = _get(causal, use_mask)

    xT = np.ascontiguousarray(x[0].T)
    cosT = np.ascontiguousarray(fc.T)  # (32, S)
    sinT = np.ascontiguousarray(fs.T)
    cc = np.tile(cosT, (4, 1))
    ssgn = np.concatenate([-sinT, sinT, -sinT, sinT], axis=0)
    kl = np.arange(128)[:, None]
    qq = np.arange(512)[None, :]
    tri = np.stack([(qq >= 128 * v + kl) for v in range(4)]).astype(np.float32)
    tri = np.ascontiguousarray(tri.reshape(512, 512))

    Wq_h = Wq.reshape(DIM, H, D)
    bq_h = bq.reshape(H, D)
    Wk_h = Wk.reshape(DIM, KVH, D)
    bk_h = bk.reshape(KVH, D)

    in_maps = []
    for c in range(NCORES):
        hs = slice(HQ * c, HQ * (c + 1))
        wq_c = _perm_eo(Wq_h[:, hs, :].transpose(0, 1, 2)).reshape(DIM, CQ)
        bq_c = _perm_eo(bq_h[hs, :]).reshape(CQ)
        wk_c = _perm_eo(Wk_h[:, c, :])
        bk_c = _perm_eo(bk_h[c, :])
        wv_c = Wv[:, 64 * c:64 * (c + 1)]
        bv_c = bv[64 * c:64 * (c + 1)]
        wkv_c = np.ascontiguousarray(np.concatenate([wv_c, wk_c], axis=1))
        bkv_c = np.concatenate([bv_c, bk_c])
        wo_c = np.ascontiguousarray(Wo[CQ * c:CQ * (c + 1), :])
        im = {
            "xT": xT, "wq": np.ascontiguousarray(wq_c), "wkv": wkv_c,
            "wo": wo_c, "bq": np.ascontiguousarray(bq_c),
            "bkv": np.ascontiguousarray(bkv_c), "cc": np.ascontiguousarray(cc),
            "ssgn": np.ascontiguousarray(ssgn), "trimask": tri,
            "ones": np.ones(128, dtype=np.float32),
        }
        if use_mask:
            im["maskt"] = np.ascontiguousarray(m2.T.astype(np.float32))
        in_maps.append(im)

    global LAST_EXEC_NS, LAST_RES
    res = run_bass_kernel_spmd(nc, in_maps, core_ids=list(range(NCORES)), trace=TRACE)
    LAST_EXEC_NS = res.exec_time_ns
    LAST_RES = res
    out = np.zeros((S, DIM), dtype=np.float32)
    for rr in res.results:
        out += rr["partial"]
    out += bo
    return out.reshape(1, S, DIM)
